# revision 13
# baseline (speedup 1.0000x reference)
"""Trainium2 Bass kernel for nn_LookupTableMy (embedding gathers + LSTM + window dots).

Computation (per sample b):
  e1 = emb[input1[b]]                 # [19, 128]
  h  = LSTM(e1)  (H=384, 19 steps)    # final hidden [384]
  e2 = emb[input2[b]]                 # [20, 128]
  s_j[k] = h[128j:128j+128] . e2[k]   # j=0..2, k=0..19
  rs[n]  = s_0[n] + s_1[n+1] + s_2[n+2]   n=0..17
  ms     = max_n rs[n]
  out    = log_softmax(ms * lin_w[:,0] + lin_b)   # [2]

Sharding: data-parallel over batch: 4096 samples -> 8 cores x 512.
v3 vs baseline:
  - all matmul operands fp16 (f32r streamed at half rate on HW)
  - emb uploaded as fp16: gather bytes halved, no e2 cast at the tail
  - gathers into dedicated tiles (no WAR coupling with the PE pipeline);
    e1 first then e2 -- the per-descriptor-limited gather stream (~1.4us
    per 128-row op) stays ahead of the LSTM and e2 lands before the tail
  - tanh(c) batched into one ACT op per step; PE clock-gate warmed during
    the fill; exp table preloaded before the tail
  - tail: DVE muls + direct 2x-mode reduces alternating gpsimd/DVE
"""

import sys
from contextlib import ExitStack

for _p in ("/opt/trn_rl_repo",):
    if _p not in sys.path:
        sys.path.insert(0, _p)

import numpy as np

import concourse.bass as bass
import concourse.tile as tile
import concourse.bacc as bacc
import concourse.mybir as mybir
from concourse import bass_utils
from concourse.bass import IndirectOffsetOnAxis
from concourse.masks import make_identity

F32 = mybir.dt.float32
F16 = mybir.dt.float16
I32 = mybir.dt.int32
AF = mybir.ActivationFunctionType
ALU = mybir.AluOpType
AX = mybir.AxisListType

V, D, OUT = 100000, 128, 2
H = 3 * D
B, L1, L2 = 4096, 19, 20
NWIN = 18
NCORES = 8
BC = B // NCORES          # 512 samples per core
NB = BC // 128            # 4 batch chunks of 128
G = 4 * H // 128          # 12 gate chunks
NJ = 3                    # hidden segments of 128

NQ = 1                    # SWDGE queues (multi-queue gave no gather speedup)
LAG = 1                   # waves of x-gather/transpose lead over LSTM steps

_cache = {}


def _emit_step(nc, t, xT, hT, cT, wt_sb, bias_sb, psum_z, gates, hcpool, tmp):
    """One LSTM step, gate-major. x/h fp16, PSUM f32, gates/c/h fp16.
    (i,f,g) chunks first so the c chain + tanh overlap the o-chunk matmuls;
    h_j lands right after o_j's activation.
    Returns (new_h, new_c): new_h = [128,512] tile per j; new_c one [128,3*512]."""

    def zmm(gc, name):
        ps = psum_z.tile([128, BC], F32, tag="zps", name=name)
        nc.tensor.matmul(
            out=ps[:],
            lhsT=wt_sb[:, 0, gc * 128 : (gc + 1) * 128],
            rhs=xT[t][:],
            start=True,
            stop=(t == 0),
        )
        if t > 0:
            for kj in range(NJ):
                nc.tensor.matmul(
                    out=ps[:],
                    lhsT=wt_sb[:, 1 + kj, gc * 128 : (gc + 1) * 128],
                    rhs=hT[kj][:],
                    start=False,
                    stop=(kj == NJ - 1),
                )
        return ps

    cn = hcpool.tile([128, NJ * BC], F16, tag="c", name=f"c{t}")
    for j in range(NJ):
        gate_t = []
        for q in range(3):  # i, f, g
            gc = q * NJ + j
            ps = zmm(gc, f"z{t}_{gc}")
            gsb = gates.tile([128, BC], F16, tag="gate", name=f"g{t}_{gc}")
            nc.scalar.activation(
                out=gsb[:],
                in_=ps[:],
                func=AF.Tanh if q == 2 else AF.Sigmoid,
                bias=bias_sb[:, gc : gc + 1],
                scale=1.0,
            )
            gate_t.append(gsb)

        cj = cn[:, j * BC : (j + 1) * BC]
        if t == 0:
            nc.vector.tensor_tensor(
                out=cj, in0=gate_t[0][:], in1=gate_t[2][:], op=ALU.mult
            )
        else:
            ig = tmp.tile([128, BC], F16, tag="ig", name=f"ig{t}_{j}")
            nc.vector.tensor_tensor(
                out=ig[:], in0=gate_t[0][:], in1=gate_t[2][:], op=ALU.mult
            )
            nc.vector.tensor_tensor(
                out=cj, in0=gate_t[1][:], in1=cT[:, j * BC : (j + 1) * BC],
                op=ALU.mult,
            )
            nc.vector.tensor_tensor(out=cj, in0=cj, in1=ig[:], op=ALU.add)

    # batched tanh over all 3 segments; overlaps the o-chunk matmuls below.
    # On the last step, split per-j so h_j lands earlier for the tail.
    tc_t = tmp.tile([128, NJ * BC], F16, tag="tc", name=f"tc{t}")
    if t == L1 - 1:
        for j in range(NJ):
            nc.scalar.activation(
                out=tc_t[:, j * BC : (j + 1) * BC],
                in_=cn[:, j * BC : (j + 1) * BC], func=AF.Tanh,
            )
    else:
        nc.scalar.activation(out=tc_t[:], in_=cn[:], func=AF.Tanh)

    new_h = [None] * NJ
    for j in range(NJ):
        gc = 3 * NJ + j
        ps = zmm(gc, f"z{t}_{gc}")
        osb = gates.tile([128, BC], F16, tag="gate", name=f"g{t}_{gc}")
        nc.scalar.activation(
            out=osb[:],
            in_=ps[:],
            func=AF.Sigmoid,
            bias=bias_sb[:, gc : gc + 1],
            scale=1.0,
        )
        hn = hcpool.tile([128, BC], F16, tag=f"h{j}", name=f"h{t}_{j}")
        nc.vector.tensor_tensor(
            out=hn[:], in0=osb[:], in1=tc_t[:, j * BC : (j + 1) * BC],
            op=ALU.mult,
        )
        new_h[j] = hn
    return new_h, cn


def _build():
    if "nc" in _cache:
        return _cache["nc"]

    nc = bacc.Bacc(
        "TRN2",
        target_bir_lowering=False,
        debug=False,
        enable_asserts=False,
        num_devices=NCORES,
        num_swdge_queues=NQ,
    )

    emb_d = nc.dram_tensor("emb", [V, D], F16, kind="ExternalInput").ap()
    wt_d = nc.dram_tensor("wt", [4, 128, 4 * H], F16, kind="ExternalInput").ap()
    bias_d = nc.dram_tensor("bias", [G, 128], F32, kind="ExternalInput").ap()
    lwb_d = nc.dram_tensor("lwb", [1, 4], F32, kind="ExternalInput").ap()
    # idx1: [128, NB, L1] (cb outer), idx2: [128, NB, L2] (cb outer)
    idx1_d = nc.dram_tensor("idx1", [128, NB, L1], I32, kind="ExternalInput").ap()
    idx2_d = nc.dram_tensor("idx2", [128, NB, L2], I32, kind="ExternalInput").ap()
    out_d = nc.dram_tensor("out", [NB * OUT, 128], F32, kind="ExternalOutput").ap()

    with tile.TileContext(nc) as tc, ExitStack() as ctx:
        singles = ctx.enter_context(tc.tile_pool(name="singles", bufs=1))
        psum_tr = ctx.enter_context(tc.tile_pool(name="psum_tr", bufs=2, space="PSUM"))
        psum_z = ctx.enter_context(tc.tile_pool(name="psum_z", bufs=6, space="PSUM"))
        gates = ctx.enter_context(tc.tile_pool(name="gates", bufs=5))
        hcpool = ctx.enter_context(tc.tile_pool(name="hc", bufs=2))
        tmp = ctx.enter_context(tc.tile_pool(name="tmp", bufs=3))
        prodp = ctx.enter_context(tc.tile_pool(name="prodp", bufs=1))
        small = ctx.enter_context(tc.tile_pool(name="small", bufs=2))

        # ---- index tensors first: the gather stream waits on them ----
        idx1_sb = singles.tile([128, NB, L1], I32, tag="idx1")
        nc.sync.dma_start(out=idx1_sb[:], in_=idx1_d)
        idx2_sb = singles.tile([128, NB, L2], I32, tag="idx2")
        nc.sync.dma_start(out=idx2_sb[:], in_=idx2_d)
        wt_sb = singles.tile([128, 4, 4 * H], F16, tag="wt")
        nc.sync.dma_start(out=wt_sb[:], in_=wt_d.rearrange("c p g -> p c g"))
        bias_sb = singles.tile([128, G], F32, tag="bias")
        nc.sync.dma_start(out=bias_sb[:], in_=bias_d.rearrange("g p -> p g"))
        lwb_sb = singles.tile([128, 4], F32, tag="lwb")
        nc.sync.dma_start(out=lwb_sb[:], in_=lwb_d.to_broadcast([128, 4]))

        ident_f = singles.tile([128, 128], F32, tag="identf")
        make_identity(nc, ident_f[:])
        ident_h = singles.tile([128, 128], F16, tag="identh")
        nc.vector.tensor_copy(out=ident_h[:], in_=ident_f[:])

        # PE clock-gate keepalive: dummy transposes emitted into the early
        # waves fill the gather-wait bubbles so the HAM stays at 8/8
        warm_ps = psum_tr.tile([128, 128], F16, tag="trps", name="warmps")

        def keepalive(n):
            for _ in range(n):
                nc.tensor.transpose(
                    out=warm_ps[:], in_=ident_h[:], identity=ident_h[:]
                )

        keepalive(40)

        # ---- gather stream: 128-row ops (one per (t,cb) / (cb,k)), e1 and e2
        # interleaved so e2 hides under the LSTM and e1 arrives just in time.
        xg = [
            [
                singles.tile([128, D], F16, tag=f"xg{t}_{cb}", name=f"xg{t}_{cb}")
                for cb in range(NB)
            ]
            for t in range(L1)
        ]
        g2 = [
            singles.tile([128, L2, D], F16, tag=f"g2_{cb}", name=f"g2_{cb}")
            for cb in range(NB)
        ]
        # all of e1 first (the LSTM consumes it slower than it arrives),
        # then e2 (done well before the tail needs it)
        for t in range(L1):
            for cb in range(NB):
                nc.gpsimd.indirect_dma_start(
                    out=xg[t][cb][:],
                    out_offset=None,
                    in_=emb_d,
                    in_offset=IndirectOffsetOnAxis(ap=idx1_sb[:, cb, t : t + 1],
                                                   axis=0),
                )
        for cb in range(NB):
            for k in range(L2):
                nc.gpsimd.indirect_dma_start(
                    out=g2[cb][:, k, :],
                    out_offset=None,
                    in_=emb_d,
                    in_offset=IndirectOffsetOnAxis(ap=idx2_sb[:, cb, k : k + 1],
                                                   axis=0),
                )

        xT = [
            singles.tile([128, BC], F16, tag=f"xT{t}", name=f"xT{t}")
            for t in range(L1)
        ]

        def transposes(w):
            for cb in range(NB):
                if w < 2:
                    keepalive(12)
                elif w < 6:
                    keepalive(8)
                elif w < 9:
                    keepalive(2)
                ps = psum_tr.tile([128, 128], F16, tag="trps", name=f"tp{w}_{cb}")
                nc.tensor.transpose(
                    out=ps[:],
                    in_=xg[w][cb][:],
                    identity=ident_h[:],
                )
                nc.vector.tensor_copy(
                    out=xT[w][:, cb * 128 : (cb + 1) * 128], in_=ps[:]
                )

        # step 0 runs right after wave 0 (one-off LAG=0); steps 1+ trail the
        # transposes by one wave so the xT copies stay off the critical path
        hT = [None] * NJ
        cT = None
        transposes(0)
        hT, cT = _emit_step(
            nc, 0, xT, hT, cT, wt_sb, bias_sb, psum_z, gates, hcpool, tmp
        )
        transposes(1)
        for w in range(2, L1 + 1):
            if w < L1:
                transposes(w)
            t = w - 1
            hT, cT = _emit_step(
                nc, t, xT, hT, cT, wt_sb, bias_sb, psum_z, gates, hcpool, tmp
            )

        # preload the exp/ln ACT table while the tail spins up (one-off ~2.7us;
        # Ln is the function that forces the natural_log_exp set switch)
        dummy_e = small.tile([128, 1], F32, tag="dummy_e", name="dummy_e")
        nc.scalar.activation(out=dummy_e[:], in_=lwb_sb[:, 0:1], func=AF.Ln,
                             scale=0.0, bias=1.0)
        nc.scalar.activation(out=dummy_e[:], in_=lwb_sb[:, 0:1], func=AF.Exp,
                             scale=0.0, bias=0.0)

        # ---- final h transposes (cb-major so cb0's dots start first) ----
        h_bmb = [
            singles.tile([128, H], F16, tag=f"hbm{cb}", name=f"hbm{cb}")
            for cb in range(NB)
        ]
        for cb in range(NB):
            for j in range(NJ):
                ps = psum_tr.tile([128, 128], F16, tag="trps", name=f"htp{j}_{cb}")
                nc.tensor.transpose(
                    out=ps[:],
                    in_=hT[j][:, cb * 128 : (cb + 1) * 128],
                    identity=ident_h[:],
                )
                nc.vector.tensor_copy(
                    out=h_bmb[cb][:, j * 128 : (j + 1) * 128], in_=ps[:]
                )

        # ---- window dots: rs[b,n] = h[b,:384] . e2[b,n:n+3,:] ----
        # per cb: ONE stacked windowed multiply [128, 18, 3, 128] (overlapping
        # windows via stride-D on both n and j; h broadcast over n). The
        # reduction runs on DVE (halving tree + XY reduce) for two cbs and on
        # the otherwise-idle ACT engine (Copy + per-window accumulate) for the
        # other two, in parallel with the remaining DVE muls.
        ACT_CBS = (1,)
        MUL_ORDER = (1, 0, 2, 3)
        rs_t = [None] * NB
        prod_t = [None] * NB
        for cb in MUL_ORDER:
            rs = small.tile([128, NWIN], F32, tag=f"rs{cb}", name=f"rs{cb}")
            rs_t[cb] = rs
            g2b = g2[cb][:]
            g2w = bass.AP(
                tensor=g2b.tensor,
                offset=g2b.offset,
                ap=[g2b.ap[0], [D, NWIN], [D, NJ], [1, D]],
            )
            hb = h_bmb[cb][:]
            h3 = bass.AP(
                tensor=hb.tensor,
                offset=hb.offset,
                ap=[hb.ap[0], [0, NWIN], [D, NJ], [1, D]],
            )
            prod = prodp.tile([128, NWIN, NJ, D], F16, tag=f"pw{cb}",
                              name=f"pw{cb}")
            prod_t[cb] = prod
            nc.vector.tensor_tensor(out=prod[:], in0=g2w, in1=h3, op=ALU.mult)
            if cb in ACT_CBS:
                junk = prodp.tile([128, NJ * D], F16, tag="junk", name=f"jk{cb}")
                for n in range(NWIN):
                    nc.scalar.activation(
                        out=junk[:],
                        in_=prod[:, n, :, :],
                        func=AF.Copy,
                        accum_out=rs[:, n : n + 1],
                    )
        for cb in MUL_ORDER:
            if cb in ACT_CBS:
                continue
            prod = prod_t[cb]
            rs = rs_t[cb]
            h1 = prodp.tile([128, NWIN, NJ, D // 2], F16, tag="h1",
                            name=f"h1_{cb}")
            nc.vector.tensor_tensor(
                out=h1[:], in0=prod[:, :, :, 0 : D // 2],
                in1=prod[:, :, :, D // 2 : D], op=ALU.add
            )
            h2 = prodp.tile([128, NWIN, NJ, D // 4], F16, tag="h2",
                            name=f"h2_{cb}")
            nc.vector.tensor_tensor(
                out=h2[:], in0=h1[:, :, :, 0 : D // 4],
                in1=h1[:, :, :, D // 4 : D // 2], op=ALU.add
            )
            h3 = prodp.tile([128, NWIN, NJ, D // 8], F16, tag="h3",
                            name=f"h3_{cb}")
            nc.vector.tensor_tensor(
                out=h3[:], in0=h2[:, :, :, 0 : D // 8],
                in1=h2[:, :, :, D // 8 : D // 4], op=ALU.add
            )
            nc.vector.tensor_reduce(
                out=rs[:], in_=h3[:], axis=AX.XY, op=ALU.add
            )

        # ---- windows max + log-softmax (exp's batched, then ln's) ----
        a_t, negm_t, se_t, lse_t = [], [], [], []
        for cb in range(NB):
            ms = small.tile([128, 1], F32, tag="ms", name=f"ms{cb}")
            nc.vector.tensor_reduce(
                out=ms[:], in_=rs_t[cb][:], axis=AX.X, op=ALU.max
            )
            a = small.tile([128, OUT], F32, tag=f"a{cb}", name=f"a{cb}")
            nc.vector.scalar_tensor_tensor(
                out=a[:],
                in0=lwb_sb[:, 0:2],
                scalar=ms[:, 0:1],
                in1=lwb_sb[:, 2:4],
                op0=ALU.mult,
                op1=ALU.add,
            )
            negm = small.tile([128, 1], F32, tag=f"negm{cb}", name=f"negm{cb}")
            nc.vector.tensor_reduce(
                out=negm[:], in_=a[:], axis=AX.X, op=ALU.max, negate=True
            )
            a_t.append(a)
            negm_t.append(negm)
        for cb in range(NB):
            e = small.tile([128, OUT], F32, tag=f"e{cb}", name=f"e{cb}")
            se = small.tile([128, 1], F32, tag=f"se{cb}", name=f"se{cb}")
            nc.scalar.activation(
                out=e[:], in_=a_t[cb][:], func=AF.Exp, bias=negm_t[cb][:, 0:1],
                accum_out=se[:],
            )
            se_t.append(se)
        for cb in range(NB):
            lse = small.tile([128, 1], F32, tag=f"lse{cb}", name=f"lse{cb}")
            nc.scalar.activation(out=lse[:], in_=se_t[cb][:], func=AF.Ln)
            lse_t.append(lse)
        # assemble [128, NB*OUT], PE-transpose to [NB*OUT, 128] so the DRAM
        # write is 8 contiguous 512B rows (host un-transposes)
        ot = small.tile([128, NB * OUT], F32, tag="ot", name="ot")
        for cb in range(NB):
            combo = small.tile([128, 1], F32, tag=f"combo{cb}", name=f"combo{cb}")
            nc.vector.tensor_tensor(
                out=combo[:], in0=negm_t[cb][:], in1=lse_t[cb][:], op=ALU.subtract
            )
            nc.vector.tensor_scalar_add(
                ot[:, cb * OUT : (cb + 1) * OUT], a_t[cb][:], combo[:, 0:1]
            )
        otp = psum_z.tile([128, BC], F32, tag="zps", name="otp")
        nc.tensor.transpose(
            out=otp[0 : NB * OUT, 0:128],
            in_=ot[:],
            identity=ident_f[:],
        )
        ott = small.tile([NB * OUT, 128], F32, tag="ott", name="ott")
        nc.vector.tensor_copy(out=ott[:], in_=otp[0 : NB * OUT, 0:128])
        nc.sync.dma_start(out=out_d, in_=ott[:])

    nc.compile()
    _cache["nc"] = nc
    return nc


def kernel(input1, input2, emb, W_ih, W_hh, b_ih, b_hh, lin_w, lin_b, _trace=False):
    input1 = np.asarray(input1, dtype=np.int64).astype(np.int32)
    input2 = np.asarray(input2, dtype=np.int64).astype(np.int32)
    emb16 = np.ascontiguousarray(np.asarray(emb, dtype=np.float32).astype(np.float16))
    W_ih = np.asarray(W_ih, dtype=np.float32)
    W_hh = np.asarray(W_hh, dtype=np.float32)
    b = np.asarray(b_ih, dtype=np.float32) + np.asarray(b_hh, dtype=np.float32)
    lin_w = np.asarray(lin_w, dtype=np.float32)
    lin_b = np.asarray(lin_b, dtype=np.float32)

    wfull = np.concatenate([W_ih, W_hh], axis=1)          # [1536, 512]
    wt = np.ascontiguousarray(wfull.T.reshape(4, 128, 4 * H).astype(np.float16))
    bias = np.ascontiguousarray(b.reshape(G, 128))
    lwb = np.ascontiguousarray(
        np.array([[lin_w[0, 0], lin_w[1, 0], lin_b[0], lin_b[1]]], dtype=np.float32)
    )

    nc = _build()

    in_maps = []
    for c in range(NCORES):
        blk1 = input1[c * BC : (c + 1) * BC].reshape(NB, 128, L1)
        blk2 = input2[c * BC : (c + 1) * BC].reshape(NB, 128, L2)
        i1 = blk1.transpose(1, 0, 2)   # [128, NB, L1]
        i2 = blk2.transpose(1, 0, 2)   # [128, NB, L2]
        in_maps.append(
            {
                "emb": emb16,
                "wt": wt,
                "bias": bias,
                "lwb": lwb,
                "idx1": np.ascontiguousarray(i1),
                "idx2": np.ascontiguousarray(i2),
            }
        )

    res = bass_utils.run_bass_kernel_spmd(
        nc, in_maps, core_ids=list(range(NCORES)), trace=_trace
    )
    if _trace:
        kernel.last_results = res
    outs = []
    for c in range(NCORES):
        r = res.results[c]["out"].reshape(NB, OUT, 128)
        outs.append(np.ascontiguousarray(r.transpose(0, 2, 1).reshape(BC, OUT)))
    return np.concatenate(outs, axis=0)


# revision 14
# speedup vs baseline: 1.0089x; 1.0089x over previous
"""Trainium2 Bass kernel for nn_LookupTableMy (embedding gathers + LSTM + window dots).

Computation (per sample b):
  e1 = emb[input1[b]]                 # [19, 128]
  h  = LSTM(e1)  (H=384, 19 steps)    # final hidden [384]
  e2 = emb[input2[b]]                 # [20, 128]
  s_j[k] = h[128j:128j+128] . e2[k]   # j=0..2, k=0..19
  rs[n]  = s_0[n] + s_1[n+1] + s_2[n+2]   n=0..17
  ms     = max_n rs[n]
  out    = log_softmax(ms * lin_w[:,0] + lin_b)   # [2]

Sharding: data-parallel over batch: 4096 samples -> 8 cores x 512.
v3 vs baseline:
  - all matmul operands fp16 (f32r streamed at half rate on HW)
  - emb uploaded as fp16: gather bytes halved, no e2 cast at the tail
  - gathers into dedicated tiles (no WAR coupling with the PE pipeline);
    e1 first then e2 -- the per-descriptor-limited gather stream (~1.4us
    per 128-row op) stays ahead of the LSTM and e2 lands before the tail
  - tanh(c) batched into one ACT op per step; PE clock-gate warmed during
    the fill; exp table preloaded before the tail
  - tail: DVE muls + direct 2x-mode reduces alternating gpsimd/DVE
"""

import sys
from contextlib import ExitStack

for _p in ("/opt/trn_rl_repo",):
    if _p not in sys.path:
        sys.path.insert(0, _p)

import numpy as np

import concourse.bass as bass
import concourse.tile as tile
import concourse.bacc as bacc
import concourse.mybir as mybir
from concourse import bass_utils
from concourse.bass import IndirectOffsetOnAxis
from concourse.masks import make_identity

F32 = mybir.dt.float32
F16 = mybir.dt.float16
I32 = mybir.dt.int32
AF = mybir.ActivationFunctionType
ALU = mybir.AluOpType
AX = mybir.AxisListType

V, D, OUT = 100000, 128, 2
H = 3 * D
B, L1, L2 = 4096, 19, 20
NWIN = 18
NCORES = 8
BC = B // NCORES          # 512 samples per core
NB = BC // 128            # 4 batch chunks of 128
G = 4 * H // 128          # 12 gate chunks
NJ = 3                    # hidden segments of 128

NQ = 1                    # SWDGE queues (multi-queue gave no gather speedup)
LAG = 1                   # waves of x-gather/transpose lead over LSTM steps

_cache = {}


def _emit_step(nc, t, xT, hT, cT, wt_sb, bias_sb, psum_z, gates, hcpool, tmp):
    """One LSTM step, gate-major. x/h fp16, PSUM f32, gates/c/h fp16.
    (i,f,g) chunks first so the c chain + tanh overlap the o-chunk matmuls;
    h_j lands right after o_j's activation.
    Returns (new_h, new_c): new_h = [128,512] tile per j; new_c one [128,3*512]."""

    def zmm(gc, name):
        ps = psum_z.tile([128, BC], F32, tag="zps", name=name)
        nc.tensor.matmul(
            out=ps[:],
            lhsT=wt_sb[:, 0, gc * 128 : (gc + 1) * 128],
            rhs=xT[t][:],
            start=True,
            stop=(t == 0),
        )
        if t > 0:
            for kj in range(NJ):
                nc.tensor.matmul(
                    out=ps[:],
                    lhsT=wt_sb[:, 1 + kj, gc * 128 : (gc + 1) * 128],
                    rhs=hT[kj][:],
                    start=False,
                    stop=(kj == NJ - 1),
                )
        return ps

    cn = hcpool.tile([128, NJ * BC], F16, tag="c", name=f"c{t}")
    for j in range(NJ):
        gate_t = []
        for q in range(3):  # i, f, g
            gc = q * NJ + j
            ps = zmm(gc, f"z{t}_{gc}")
            gsb = gates.tile([128, BC], F16, tag="gate", name=f"g{t}_{gc}")
            nc.scalar.activation(
                out=gsb[:],
                in_=ps[:],
                func=AF.Tanh if q == 2 else AF.Sigmoid,
                bias=bias_sb[:, gc : gc + 1],
                scale=1.0,
            )
            gate_t.append(gsb)

        cj = cn[:, j * BC : (j + 1) * BC]
        if t == 0:
            nc.vector.tensor_tensor(
                out=cj, in0=gate_t[0][:], in1=gate_t[2][:], op=ALU.mult
            )
        else:
            ig = tmp.tile([128, BC], F16, tag="ig", name=f"ig{t}_{j}")
            nc.vector.tensor_tensor(
                out=ig[:], in0=gate_t[0][:], in1=gate_t[2][:], op=ALU.mult
            )
            nc.vector.tensor_tensor(
                out=cj, in0=gate_t[1][:], in1=cT[:, j * BC : (j + 1) * BC],
                op=ALU.mult,
            )
            nc.vector.tensor_tensor(out=cj, in0=cj, in1=ig[:], op=ALU.add)

    # batched tanh over all 3 segments; overlaps the o-chunk matmuls below.
    # On the last step, split per-j so h_j lands earlier for the tail.
    tc_t = tmp.tile([128, NJ * BC], F16, tag="tc", name=f"tc{t}")
    if t == L1 - 1:
        for j in range(NJ):
            nc.scalar.activation(
                out=tc_t[:, j * BC : (j + 1) * BC],
                in_=cn[:, j * BC : (j + 1) * BC], func=AF.Tanh,
            )
    else:
        nc.scalar.activation(out=tc_t[:], in_=cn[:], func=AF.Tanh)

    new_h = [None] * NJ
    for j in range(NJ):
        gc = 3 * NJ + j
        ps = zmm(gc, f"z{t}_{gc}")
        osb = gates.tile([128, BC], F16, tag="gate", name=f"g{t}_{gc}")
        nc.scalar.activation(
            out=osb[:],
            in_=ps[:],
            func=AF.Sigmoid,
            bias=bias_sb[:, gc : gc + 1],
            scale=1.0,
        )
        hn = hcpool.tile([128, BC], F16, tag=f"h{j}", name=f"h{t}_{j}")
        nc.vector.tensor_tensor(
            out=hn[:], in0=osb[:], in1=tc_t[:, j * BC : (j + 1) * BC],
            op=ALU.mult,
        )
        new_h[j] = hn
    return new_h, cn


def _build():
    if "nc" in _cache:
        return _cache["nc"]

    nc = bacc.Bacc(
        "TRN2",
        target_bir_lowering=False,
        debug=False,
        enable_asserts=False,
        num_devices=NCORES,
        num_swdge_queues=NQ,
    )

    emb_d = nc.dram_tensor("emb", [V, D], F16, kind="ExternalInput").ap()
    wt_d = nc.dram_tensor("wt", [4, 128, 4 * H], F16, kind="ExternalInput").ap()
    bias_d = nc.dram_tensor("bias", [G, 128], F32, kind="ExternalInput").ap()
    lwb_d = nc.dram_tensor("lwb", [1, 4], F32, kind="ExternalInput").ap()
    # idx1: [128, NB, L1] (cb outer), idx2: [128, NB, L2] (cb outer)
    idx1_d = nc.dram_tensor("idx1", [128, NB, L1], I32, kind="ExternalInput").ap()
    idx2_d = nc.dram_tensor("idx2", [128, NB, L2], I32, kind="ExternalInput").ap()
    out_d = nc.dram_tensor("out", [NB * OUT, 128], F32, kind="ExternalOutput").ap()

    with tile.TileContext(nc) as tc, ExitStack() as ctx:
        singles = ctx.enter_context(tc.tile_pool(name="singles", bufs=1))
        psum_tr = ctx.enter_context(tc.tile_pool(name="psum_tr", bufs=2, space="PSUM"))
        psum_z = ctx.enter_context(tc.tile_pool(name="psum_z", bufs=6, space="PSUM"))
        gates = ctx.enter_context(tc.tile_pool(name="gates", bufs=5))
        hcpool = ctx.enter_context(tc.tile_pool(name="hc", bufs=2))
        tmp = ctx.enter_context(tc.tile_pool(name="tmp", bufs=3))
        prodp = ctx.enter_context(tc.tile_pool(name="prodp", bufs=1))
        small = ctx.enter_context(tc.tile_pool(name="small", bufs=2))

        # ---- index tensors first: the gather stream waits on them ----
        idx1_sb = singles.tile([128, NB, L1], I32, tag="idx1")
        nc.sync.dma_start(out=idx1_sb[:], in_=idx1_d)
        idx2_sb = singles.tile([128, NB, L2], I32, tag="idx2")
        nc.sync.dma_start(out=idx2_sb[:], in_=idx2_d)
        wt_sb = singles.tile([128, 4, 4 * H], F16, tag="wt")
        nc.sync.dma_start(out=wt_sb[:], in_=wt_d.rearrange("c p g -> p c g"))
        bias_sb = singles.tile([128, G], F32, tag="bias")
        nc.sync.dma_start(out=bias_sb[:], in_=bias_d.rearrange("g p -> p g"))
        lwb_sb = singles.tile([128, 4], F32, tag="lwb")
        nc.sync.dma_start(out=lwb_sb[:], in_=lwb_d.to_broadcast([128, 4]))

        ident_f = singles.tile([128, 128], F32, tag="identf")
        make_identity(nc, ident_f[:])
        ident_h = singles.tile([128, 128], F16, tag="identh")
        nc.vector.tensor_copy(out=ident_h[:], in_=ident_f[:])

        # PE clock-gate keepalive: dummy transposes emitted into the early
        # waves fill the gather-wait bubbles so the HAM stays at 8/8
        warm_ps = psum_tr.tile([128, 128], F16, tag="trps", name="warmps")

        def keepalive(n):
            for _ in range(n):
                nc.tensor.transpose(
                    out=warm_ps[:], in_=ident_h[:], identity=ident_h[:]
                )

        keepalive(40)

        # ---- gather stream: 128-row ops (one per (t,cb) / (cb,k)), e1 and e2
        # interleaved so e2 hides under the LSTM and e1 arrives just in time.
        xg = [
            [
                singles.tile([128, D], F16, tag=f"xg{t}_{cb}", name=f"xg{t}_{cb}")
                for cb in range(NB)
            ]
            for t in range(L1)
        ]
        g2 = [
            singles.tile([128, L2, D], F16, tag=f"g2_{cb}", name=f"g2_{cb}")
            for cb in range(NB)
        ]
        # all of e1 first (the LSTM consumes it slower than it arrives),
        # then e2 (done well before the tail needs it)
        for t in range(L1):
            for cb in range(NB):
                nc.gpsimd.indirect_dma_start(
                    out=xg[t][cb][:],
                    out_offset=None,
                    in_=emb_d,
                    in_offset=IndirectOffsetOnAxis(ap=idx1_sb[:, cb, t : t + 1],
                                                   axis=0),
                )
        for cb in range(NB):
            for k in range(L2):
                nc.gpsimd.indirect_dma_start(
                    out=g2[cb][:, k, :],
                    out_offset=None,
                    in_=emb_d,
                    in_offset=IndirectOffsetOnAxis(ap=idx2_sb[:, cb, k : k + 1],
                                                   axis=0),
                )

        xT = [
            singles.tile([128, BC], F16, tag=f"xT{t}", name=f"xT{t}")
            for t in range(L1)
        ]

        def transposes(w):
            for cb in range(NB):
                if w < 2:
                    keepalive(12)
                elif w < 6:
                    keepalive(8)
                elif w < 9:
                    keepalive(2)
                ps = psum_tr.tile([128, 128], F16, tag="trps", name=f"tp{w}_{cb}")
                nc.tensor.transpose(
                    out=ps[:],
                    in_=xg[w][cb][:],
                    identity=ident_h[:],
                )
                nc.vector.tensor_copy(
                    out=xT[w][:, cb * 128 : (cb + 1) * 128], in_=ps[:]
                )

        # step 0 runs right after wave 0 (one-off LAG=0); steps 1+ trail the
        # transposes by one wave so the xT copies stay off the critical path
        hT = [None] * NJ
        cT = None
        transposes(0)
        hT, cT = _emit_step(
            nc, 0, xT, hT, cT, wt_sb, bias_sb, psum_z, gates, hcpool, tmp
        )
        transposes(1)
        for w in range(2, L1 + 1):
            if w < L1:
                transposes(w)
            t = w - 1
            hT, cT = _emit_step(
                nc, t, xT, hT, cT, wt_sb, bias_sb, psum_z, gates, hcpool, tmp
            )

        # preload the exp/ln ACT table while the tail spins up (one-off ~2.7us;
        # Ln is the function that forces the natural_log_exp set switch)
        dummy_e = small.tile([128, 1], F32, tag="dummy_e", name="dummy_e")
        nc.scalar.activation(out=dummy_e[:], in_=lwb_sb[:, 0:1], func=AF.Ln,
                             scale=0.0, bias=1.0)
        nc.scalar.activation(out=dummy_e[:], in_=lwb_sb[:, 0:1], func=AF.Exp,
                             scale=0.0, bias=0.0)

        # ---- final h transposes (cb-major so cb0's dots start first) ----
        h_bmb = [
            singles.tile([128, H], F16, tag=f"hbm{cb}", name=f"hbm{cb}")
            for cb in range(NB)
        ]
        for cb in range(NB):
            for j in range(NJ):
                ps = psum_tr.tile([128, 128], F16, tag="trps", name=f"htp{j}_{cb}")
                nc.tensor.transpose(
                    out=ps[:],
                    in_=hT[j][:, cb * 128 : (cb + 1) * 128],
                    identity=ident_h[:],
                )
                nc.vector.tensor_copy(
                    out=h_bmb[cb][:, j * 128 : (j + 1) * 128], in_=ps[:]
                )

        # ---- window dots: rs[b,n] = h[b,:384] . e2[b,n:n+3,:] ----
        # per cb: ONE stacked windowed multiply [128, 18, 3, 128] (overlapping
        # windows via stride-D on both n and j; h broadcast over n). The
        # reduction runs on DVE (halving tree + XY reduce) for two cbs and on
        # the otherwise-idle ACT engine (Copy + per-window accumulate) for the
        # other two, in parallel with the remaining DVE muls.
        ACT_CBS = (1,)
        MUL_ORDER = (1, 0, 2, 3)
        rs_t = [None] * NB
        prod_t = [None] * NB
        for cb in MUL_ORDER:
            rs = small.tile([128, NWIN], F32, tag=f"rs{cb}", name=f"rs{cb}")
            rs_t[cb] = rs
            g2b = g2[cb][:]
            g2w = bass.AP(
                tensor=g2b.tensor,
                offset=g2b.offset,
                ap=[g2b.ap[0], [D, NWIN], [D, NJ], [1, D]],
            )
            hb = h_bmb[cb][:]
            h3 = bass.AP(
                tensor=hb.tensor,
                offset=hb.offset,
                ap=[hb.ap[0], [0, NWIN], [D, NJ], [1, D]],
            )
            prod = prodp.tile([128, NWIN, NJ, D], F16, tag=f"pw{cb}",
                              name=f"pw{cb}")
            prod_t[cb] = prod
            nc.vector.tensor_tensor(out=prod[:], in0=g2w, in1=h3, op=ALU.mult)
            if cb in ACT_CBS:
                junk = prodp.tile([128, NJ * D], F16, tag="junk", name=f"jk{cb}")
                for n in range(NWIN):
                    nc.scalar.activation(
                        out=junk[:],
                        in_=prod[:, n, :, :],
                        func=AF.Copy,
                        accum_out=rs[:, n : n + 1],
                    )
        for cb in MUL_ORDER:
            if cb in ACT_CBS:
                continue
            prod = prod_t[cb]
            rs = rs_t[cb]
            h1 = prodp.tile([128, NWIN, NJ, D // 2], F16, tag="h1",
                            name=f"h1_{cb}")
            nc.vector.tensor_tensor(
                out=h1[:], in0=prod[:, :, :, 0 : D // 2],
                in1=prod[:, :, :, D // 2 : D], op=ALU.add
            )
            h2 = prodp.tile([128, NWIN, NJ, D // 4], F16, tag="h2",
                            name=f"h2_{cb}")
            nc.vector.tensor_tensor(
                out=h2[:], in0=h1[:, :, :, 0 : D // 4],
                in1=h1[:, :, :, D // 4 : D // 2], op=ALU.add
            )
            nc.vector.tensor_reduce(
                out=rs[:], in_=h2[:], axis=AX.XY, op=ALU.add
            )

        # ---- windows max + log-softmax (exp's batched, then ln's) ----
        a_t, negm_t, se_t, lse_t = [], [], [], []
        for cb in range(NB):
            ms = small.tile([128, 1], F32, tag="ms", name=f"ms{cb}")
            nc.vector.tensor_reduce(
                out=ms[:], in_=rs_t[cb][:], axis=AX.X, op=ALU.max
            )
            a = small.tile([128, OUT], F32, tag=f"a{cb}", name=f"a{cb}")
            nc.vector.scalar_tensor_tensor(
                out=a[:],
                in0=lwb_sb[:, 0:2],
                scalar=ms[:, 0:1],
                in1=lwb_sb[:, 2:4],
                op0=ALU.mult,
                op1=ALU.add,
            )
            negm = small.tile([128, 1], F32, tag=f"negm{cb}", name=f"negm{cb}")
            nc.vector.tensor_reduce(
                out=negm[:], in_=a[:], axis=AX.X, op=ALU.max, negate=True
            )
            a_t.append(a)
            negm_t.append(negm)
        for cb in range(NB):
            e = small.tile([128, OUT], F32, tag=f"e{cb}", name=f"e{cb}")
            se = small.tile([128, 1], F32, tag=f"se{cb}", name=f"se{cb}")
            nc.scalar.activation(
                out=e[:], in_=a_t[cb][:], func=AF.Exp, bias=negm_t[cb][:, 0:1],
                accum_out=se[:],
            )
            se_t.append(se)
        for cb in range(NB):
            lse = small.tile([128, 1], F32, tag=f"lse{cb}", name=f"lse{cb}")
            nc.scalar.activation(out=lse[:], in_=se_t[cb][:], func=AF.Ln)
            lse_t.append(lse)
        # assemble [128, NB*OUT], PE-transpose to [NB*OUT, 128] so the DRAM
        # write is 8 contiguous 512B rows (host un-transposes)
        ot = small.tile([128, NB * OUT], F32, tag="ot", name="ot")
        for cb in range(NB):
            combo = small.tile([128, 1], F32, tag=f"combo{cb}", name=f"combo{cb}")
            nc.vector.tensor_tensor(
                out=combo[:], in0=negm_t[cb][:], in1=lse_t[cb][:], op=ALU.subtract
            )
            nc.vector.tensor_scalar_add(
                ot[:, cb * OUT : (cb + 1) * OUT], a_t[cb][:], combo[:, 0:1]
            )
        otp = psum_z.tile([128, BC], F32, tag="zps", name="otp")
        nc.tensor.transpose(
            out=otp[0 : NB * OUT, 0:128],
            in_=ot[:],
            identity=ident_f[:],
        )
        ott = small.tile([NB * OUT, 128], F32, tag="ott", name="ott")
        nc.vector.tensor_copy(out=ott[:], in_=otp[0 : NB * OUT, 0:128])
        nc.sync.dma_start(out=out_d, in_=ott[:])

    nc.compile()
    _cache["nc"] = nc
    return nc


def kernel(input1, input2, emb, W_ih, W_hh, b_ih, b_hh, lin_w, lin_b, _trace=False):
    input1 = np.asarray(input1, dtype=np.int64).astype(np.int32)
    input2 = np.asarray(input2, dtype=np.int64).astype(np.int32)
    emb16 = np.ascontiguousarray(np.asarray(emb, dtype=np.float32).astype(np.float16))
    W_ih = np.asarray(W_ih, dtype=np.float32)
    W_hh = np.asarray(W_hh, dtype=np.float32)
    b = np.asarray(b_ih, dtype=np.float32) + np.asarray(b_hh, dtype=np.float32)
    lin_w = np.asarray(lin_w, dtype=np.float32)
    lin_b = np.asarray(lin_b, dtype=np.float32)

    wfull = np.concatenate([W_ih, W_hh], axis=1)          # [1536, 512]
    wt = np.ascontiguousarray(wfull.T.reshape(4, 128, 4 * H).astype(np.float16))
    bias = np.ascontiguousarray(b.reshape(G, 128))
    lwb = np.ascontiguousarray(
        np.array([[lin_w[0, 0], lin_w[1, 0], lin_b[0], lin_b[1]]], dtype=np.float32)
    )

    nc = _build()

    in_maps = []
    for c in range(NCORES):
        blk1 = input1[c * BC : (c + 1) * BC].reshape(NB, 128, L1)
        blk2 = input2[c * BC : (c + 1) * BC].reshape(NB, 128, L2)
        i1 = blk1.transpose(1, 0, 2)   # [128, NB, L1]
        i2 = blk2.transpose(1, 0, 2)   # [128, NB, L2]
        in_maps.append(
            {
                "emb": emb16,
                "wt": wt,
                "bias": bias,
                "lwb": lwb,
                "idx1": np.ascontiguousarray(i1),
                "idx2": np.ascontiguousarray(i2),
            }
        )

    res = bass_utils.run_bass_kernel_spmd(
        nc, in_maps, core_ids=list(range(NCORES)), trace=_trace
    )
    if _trace:
        kernel.last_results = res
    outs = []
    for c in range(NCORES):
        r = res.results[c]["out"].reshape(NB, OUT, 128)
        outs.append(np.ascontiguousarray(r.transpose(0, 2, 1).reshape(BC, OUT)))
    return np.concatenate(outs, axis=0)
